# revision 50
# baseline (speedup 1.0000x reference)
"""AdjacentAttention on 8 TRN2 NeuronCores.

Strategy (all shapes hardcoded for B=1, N=10000, A=32, D=256, H=4, DH=64):

Host:
  - kv projection commutes with the neighbor gather: the device computes a
    kv table (x @ Wkv, bf16) once and gathers *projected* rows.
  - ~50% of neighbors are masked out.  The host compacts each node's
    neighbor list to its valid entries, sorts nodes by degree, and deals
    them into 10 degree-homogeneous tile groups of 1024 (128 nodes x 8
    cores), so each tile only gathers/computes its group-max degree a_t.
  - DEMAND-ORDERED kv table: each core's kv-table rows are permuted into
    first-use order over the emission sequence of its 10 node tiles, so
    tile t's entire gather only needs the first C_t rows of the table.
    Rows a core never references are not computed at all.
  - The v half of Wkv's columns (and null_v, and Wo's rows) are permuted
    from (h, dh) to (dh, h) order so the attention-weighted v multiply can
    broadcast attn over the *middle* axis (DVE supports stride-0 middle
    broadcast, not inner), removing the big attn broadcast copy.
  - x is passed per-core, pre-transposed (bf16), chunk-major in the
    demand order; attention-scale is folded into Wq.

Device (SPMD, identical program per core, per-core data).  Emission order
is the per-engine schedule (streams are in-order), so the program is laid
out as an explicit software pipeline:
  - Phase B (q tiles) first on PE, then write groups of the kv table
    (phase A) interleaved with phase C by gather dependency: write groups
    are emitted just before the first gather that needs them, so the ACT
    copy stream never makes a gather wait on unrelated exp work.
  - Phase C per tile is split into qk (DVE mult + tensor_reduce), exp
    (ACT), post (DVE: wts/denoms/weighted-v/normalize into a small
    staging tile) and pe (PE transpose + out-projection).  The emission
    interleave is gather(t+2)* / qk(t+1) / exp(t+1) / post(t) / pe(t-1):
    DVE works on tile t+1's qk while ACT exp(t) is in flight, and the
    gather stream runs two tiles ahead of DVE.
  - The normalize step writes its result to a staging tile, so the big
    gather buffer is freed at the end of post(t) and the gather for tile
    t+4 (pool bufs=4) never waits on PE.
  - Gathers rotate over 4 SWDGE queues so descriptor generation never
    stalls on ring space.
"""

import os

import numpy as np
import ml_dtypes

import bass_rust
import concourse.bacc as bacc
import concourse.tile as tile
from concourse import mybir
from concourse.bass_utils import run_bass_kernel_spmd

BF = ml_dtypes.bfloat16

N, A, D, H, DH = 10000, 32, 256, 4, 64
NCORES, P, NT = 8, 128, 10
GROUP = NCORES * P            # 1024 nodes per tile-group
NPAD = NT * GROUP             # 10240
HD = H * DH                   # 256
KVW = 2 * HD                  # 512 (k|v row width, elements, bf16)

LAST_EXEC_NS = None


def _wgroup_sizes(kv_tiles):
    """Write-group sizes in 128-row tiles: small first so early gathers
    unblock fast, then 8-tile groups."""
    sizes = []
    for s in [4, 4, 6, 6]:
        if sum(sizes) + s <= kv_tiles:
            sizes.append(s)
    while sum(sizes) < kv_tiles:
        sizes.append(min(8, kv_tiles - sum(sizes)))
    return sizes


def _chunk_cuts(aa):
    """Gather chunk cuts over the aa slot columns (8-slot chunks)."""
    cuts = list(range(8, aa, 8))
    return list(zip([0] + cuts, cuts + [aa]))


def _build(a_ts, dep_groups, kv_tiles):
    """a_ts: per-tile slot count.  dep_groups[t]: highest kv write group
    tile t's gather needs.  kv_tiles: 128-row tiles in the (demand-ordered,
    trimmed) kv table.  Table row 0 is the null kv row; demand rows start
    at row 1."""
    # dynamic_dma_scratch_size: the SWDGE descriptor-ring carveout scales
    # with it; the default 16K gives ~32 descs per engine-ring, which makes
    # every 1024-descriptor gather chunk block mid-generation (serializing
    # gather transfers on GpSimd).  36K fits an 8-slot chunk per ring.
    nc = bacc.Bacc("TRN2", target_bir_lowering=False, num_swdge_queues=4,
                   dynamic_dma_scratch_size=36864)
    bf = mybir.dt.bfloat16
    f32 = mybir.dt.float32
    mult = mybir.AluOpType.mult
    add = mybir.AluOpType.add

    aas = [a + 1 for a in a_ts]
    idxcols = 8 * sum(aas)
    mcols = sum(aas)
    nkv = kv_tiles * P + 1       # last row is the null kv row
    wsizes = _wgroup_sizes(kv_tiles)
    nwg = len(wsizes)
    wstarts = [sum(wsizes[:i]) for i in range(nwg)]

    xT = nc.declare_dram_parameter("xT", [P, kv_tiles, 2, P], bf, isOutput=False)
    xpT = nc.declare_dram_parameter("xpT", [P, 2, NT * P], bf, isOutput=False)
    wq = nc.declare_dram_parameter("wq", [P, 2, HD], bf, isOutput=False)
    wkv = nc.declare_dram_parameter("wkv", [P, 2, KVW], bf, isOutput=False)
    wo = nc.declare_dram_parameter("wo", [P, 2, D], bf, isOutput=False)
    bo_p = nc.declare_dram_parameter("bo", [1, D], bf, isOutput=False)
    nullkv = nc.declare_dram_parameter("nullkv", [P, KVW], bf, isOutput=False)
    ident_p = nc.declare_dram_parameter("ident", [P, P], bf, isOutput=False)
    idxs_p = nc.declare_dram_parameter("idxs", [P, idxcols], mybir.dt.int16, isOutput=False)
    masks_p = nc.declare_dram_parameter("masks", [P, mcols], f32, isOutput=False)
    out_p = nc.declare_dram_parameter("out", [NT * P, D], f32, isOutput=True)

    # table laid out partition-major: position (p, j) holds demand row
    # j*128 + p, so a write group's DMA is one contiguous (wsize KB)
    # descriptor per partition instead of 128 1KB ones.
    kv_dram = nc.dram_tensor("kv_scratch", [nkv, KVW], bf)
    kv_pj = kv_dram[0:P * kv_tiles, :].rearrange("(p j) c -> p j c", j=kv_tiles)
    warm_dram = nc.dram_tensor("warm_scratch", [P, KVW], bf)

    with tile.TileContext(nc) as tc:
        with (
            tc.tile_pool(name="singles", bufs=1) as singles,
            tc.tile_pool(name="xchunk", bufs=2) as xchunk,
            tc.tile_pool(name="kvstage", bufs=4) as kvstage,
            tc.tile_pool(name="kvgp", bufs=3) as kvgp,
            tc.tile_pool(name="small", bufs=4) as small,
            tc.tile_pool(name="vout", bufs=3) as voutp,
            tc.tile_pool(name="work", bufs=2) as work,
            tc.tile_pool(name="psA", bufs=6, space="PSUM") as psA,
            tc.tile_pool(name="psT", bufs=1, space="PSUM") as psT,
            tc.tile_pool(name="psF", bufs=1, space="PSUM") as psF,
        ):
            # ---------- constants ----------
            # warmup idx via on-device memset (no DMA dep: the scheduler is
            # dependency-driven and would happily run the big constant loads
            # before a tiny warmup-idx load on the same queue).
            widx_sb = singles.tile([P, 8], mybir.dt.int16)
            widx_dma = nc.vector.memset(widx_sb[:], 0)
            wkv_sb = singles.tile([P, 2, KVW], bf)
            nc.sync.dma_start(out=wkv_sb[:], in_=wkv[:])
            wq_sb = singles.tile([P, 2, HD], bf)
            nc.sync.dma_start(out=wq_sb[:], in_=wq[:])
            idx_sb = singles.tile([P, idxcols], mybir.dt.int16)
            idx_dma = nc.sync.dma_start(out=idx_sb[:], in_=idxs_p[:])
            wo_sb = singles.tile([P, 2, D], bf)
            nc.sync.dma_start(out=wo_sb[:], in_=wo[:])
            bo_sb = singles.tile([1, D], bf)
            nc.sync.dma_start(out=bo_sb[:], in_=bo_p[:])
            ident_sb = singles.tile([P, P], bf)
            nc.sync.dma_start(out=ident_sb[:], in_=ident_p[:])
            mask_sb = singles.tile([P, mcols], f32)
            nc.sync.dma_start(out=mask_sb[:], in_=masks_p[:])
            ones1 = singles.tile([1, P], bf)
            nc.vector.memset(ones1[:], 1.0)

            # warmup: force the Q7 dma_gather library load + SWDGE path
            # setup on every queue before the first real gather.  Disjoint
            # destination slices so the 4 warmups don't serialize on WAW.
            warm = small.tile([P, 4, KVW], bf, tag="warm")
            for q in range(4):
                gw = nc.gpsimd.dma_gather(
                    warm[:, q:q + 1, :], warm_dram[:], widx_sb[:, 0:8],
                    num_idxs=P, num_idxs_reg=P, elem_size=KVW,
                    single_packet=False, queue_num=q)
                bass_rust.add_dep_helper(gw.ins, widx_dma.ins,
                                         reason="warmup gather reads warm idx")

            # null kv row -> last table row (1KB DRAM->DRAM)
            null_write = nc.sync.dma_start(out=kv_dram[nkv - 1:nkv, :],
                                           in_=nullkv[0:1, :])

            # ---------- phase A emitters ----------
            kv_writes = []
            xc_tiles = {}
            XCMAX = 8

            def load_xc(g):
                if g >= nwg or g in xc_tiles:
                    return
                w = wsizes[g]
                xc = xchunk.tile([P, XCMAX, 2, P], bf, tag="xc")
                nc.scalar.dma_start(out=xc[:, 0:w, :, :],
                                    in_=xT[:, wstarts[g]:wstarts[g] + w, :, :])
                xc_tiles[g] = xc

            copy_ctr = [0]

            def emit_wgroup(g):
                # single-tile PSUM pipeline (6 banks deep); PSUM->SBUF
                # copies alternate between ACT and DVE so neither engine
                # paces phase A.  The partition-major table layout makes
                # the write one big descriptor per partition.
                g0 = wstarts[g]
                nb_g = wsizes[g]
                st = kvstage.tile([P, XCMAX, KVW], bf, tag="kvstage")
                xc = xc_tiles[g]
                for i in range(nb_g):
                    ps = psA.tile([P, KVW], f32, space="PSUM", tag="psA")
                    nc.tensor.matmul(
                        out=ps[:], lhsT=xc[:, i, 0, :],
                        rhs=wkv_sb[:, 0, :], start=True, stop=False)
                    nc.tensor.matmul(
                        out=ps[:], lhsT=xc[:, i, 1, :],
                        rhs=wkv_sb[:, 1, :], start=False, stop=True)
                    if copy_ctr[0] % 2 == 0:
                        nc.scalar.copy(out=st[:, i, :], in_=ps[:])
                    else:
                        nc.vector.tensor_scalar_mul(st[:, i, :], ps[:], 1.0)
                    copy_ctr[0] += 1
                load_xc(g + 2)
                kv_writes.append(
                    nc.sync.dma_start(out=kv_pj[:, g0:g0 + nb_g, :],
                                      in_=st[:, 0:nb_g, :]))

            def emit_phase_b():
                xp_sb = singles.tile([P, 2, NT * P], bf)
                nc.gpsimd.dma_start(out=xp_sb[:], in_=xpT[:])
                q_sb = singles.tile([P, NT, HD], bf)
                for t in range(NT):
                    psq = psF.tile([P, HD], f32, space="PSUM", tag="psF")
                    nc.tensor.matmul(
                        out=psq[:], lhsT=xp_sb[:, 0, t * P:(t + 1) * P],
                        rhs=wq_sb[:, 0, :], start=True, stop=False)
                    nc.tensor.matmul(
                        out=psq[:], lhsT=xp_sb[:, 1, t * P:(t + 1) * P],
                        rhs=wq_sb[:, 1, :], start=False, stop=True)
                    nc.scalar.copy(out=q_sb[:, t, :], in_=psq[:])
                return q_sb

            # ---------- phase C: attention per tile ----------
            tile_off = []
            io = 0
            mo = 0
            for aa in aas:
                tile_off.append((io, mo))
                io += 8 * aa
                mo += aa

            kvg_map = {}
            sim_map = {}
            exp_map = {}
            vout_map = {}
            qctr = [0]

            def emit_gather(t):
                aa = a_ts[t] + 1
                io, _ = tile_off[t]
                kv_g = kvgp.tile([P, aa, KVW], bf, tag="kvg")
                kvg_map[t] = kv_g
                for (c0, c1) in _chunk_cuts(aa):
                    gi = nc.gpsimd.dma_gather(
                        kv_g[:, c0:c1, :], kv_dram[:],
                        idx_sb[:, io + 8 * c0:io + 8 * c1],
                        num_idxs=P * (c1 - c0), num_idxs_reg=P * (c1 - c0),
                        elem_size=KVW, single_packet=False,
                        queue_num=qctr[0] % 4)
                    qctr[0] += 1
                    # Tile's auto-dep tracking misses dma_gather's *input*
                    # APs (idx tile + DRAM source); add edges explicitly,
                    # to EVERY write group up to this tile's max row.
                    bass_rust.add_dep_helper(gi.ins, idx_dma.ins,
                                             reason="gather reads idx blob")
                    bass_rust.add_dep_helper(gi.ins, null_write.ins,
                                             reason="gather reads null row")
                    for wgi in range(dep_groups[t] + 1):
                        bass_rust.add_dep_helper(gi.ins, kv_writes[wgi].ins,
                                                 reason="gather reads kv prefix")

            def emit_qk(t, q_sb):
                aa = a_ts[t] + 1
                kv_g = kvg_map[t]
                # q.k multiply (bf16 2x) in-place over the k half, then a
                # halving tree over dh -> sim in k[...,0] stripes
                k4 = kv_g[:, :, 0:HD].rearrange("p a (h d) -> p a h d", d=DH)
                qb = (q_sb[:, t:t + 1, :]
                      .rearrange("p o (h d) -> p o h d", d=DH)
                      .broadcast_to([P, aa, H, DH]))
                nc.vector.tensor_tensor(out=k4, in0=k4, in1=qb, op=mult)
                w = DH
                while w > 1:
                    h2 = w // 2
                    nc.vector.tensor_tensor(
                        out=k4[:, :, :, 0:h2], in0=k4[:, :, :, 0:h2],
                        in1=k4[:, :, :, h2:w], op=add)
                    w = h2

            def emit_exp(t):
                aa = a_ts[t] + 1
                kv_g = kvg_map[t]
                exp_s = small.tile([P, aa, H], f32, tag="exp")
                exp_map[t] = exp_s
                nc.scalar.activation(
                    out=exp_s[:], in_=kv_g[:, :, 0:HD:DH],
                    func=mybir.ActivationFunctionType.Exp)

            def emit_post(t):
                aa = a_ts[t] + 1
                _, mo = tile_off[t]
                kv_g = kvg_map.pop(t)
                exp_s = exp_map.pop(t)
                # w = mask * exp (bf16 out); unnormalized weights
                wts = small.tile([P, aa, H], bf, tag="wts")
                mb = (mask_sb[:, mo:mo + aa]
                      .rearrange("p (a o) -> p a o", o=1)
                      .broadcast_to([P, aa, H]))
                nc.vector.tensor_tensor(out=wts[:], in0=exp_s[:], in1=mb, op=mult)
                denom = small.tile([P, H], f32, tag="denom")
                nc.vector.tensor_reduce(
                    out=denom[:], in_=wts[:].rearrange("p a h -> p h a"),
                    axis=mybir.AxisListType.X, op=add)
                recip = small.tile([P, H], f32, tag="recip")
                nc.vector.reciprocal(out=recip[:], in_=denom[:])

                # v half is (dh, h)-interleaved: broadcast wts over the
                # *middle* dh axis (stride-0 middle is supported on DVE)
                v4 = kv_g[:, :, HD:KVW].rearrange("p a (d h) -> p a d h", h=H)
                wb = (wts[:].rearrange("p a (o h) -> p a o h", o=1)
                      .broadcast_to([P, aa, DH, H]))
                nc.vector.tensor_tensor(out=v4, in0=v4, in1=wb, op=mult)
                vflat = kv_g[:, :, HD:KVW]          # [P, aa, 256] view
                w = aa
                while w > 1:
                    h2 = w // 2
                    nc.vector.tensor_tensor(
                        out=vflat[:, 0:h2, :], in0=vflat[:, 0:h2, :],
                        in1=vflat[:, h2:2 * h2, :], op=add)
                    if w % 2 == 1:
                        nc.vector.tensor_tensor(
                            out=vflat[:, 0:1, :], in0=vflat[:, 0:1, :],
                            in1=vflat[:, w - 1:w, :], op=add)
                    w = h2
                # normalize the 256-wide sum by 1/denom (broadcast over dh),
                # writing to a small staging tile so kv_g is freed here.
                vs = kv_g[:, 0:1, HD:KVW].rearrange("p o (d h) -> p (o d) h", h=H)
                rb = (recip[:].rearrange("p (o h) -> p o h", o=1)
                      .broadcast_to([P, DH, H]))
                vout = voutp.tile([P, DH, H], bf, tag="vout")
                vout_map[t] = vout
                nc.vector.tensor_tensor(out=vout[:], in0=vs, in1=rb, op=mult)

            # all tile outputs accumulate in SBUF; ONE DMA at the very end.
            # (A per-tile out write sits in the sync queue's static order in
            # front of later kv-table writes; its not-yet-ready semaphore
            # head-of-line blocks them on real HW, stalling every gather.)
            outf_all = singles.tile([P, NT, D], f32)

            def emit_pe(t):
                out_attn = vout_map.pop(t)[:].rearrange("p d h -> p (d h)")
                outT = work.tile([P, 2, P], bf, tag="outT")
                for j in range(2):
                    pst = psT.tile([P, P], bf, space="PSUM", tag="psT")
                    nc.tensor.transpose(
                        out=pst[:], in_=out_attn[:, j * P:(j + 1) * P],
                        identity=ident_sb[:])
                    nc.scalar.copy(out=outT[:, j, :], in_=pst[:])

                psf = psF.tile([P, D], f32, space="PSUM", tag="psF")
                nc.tensor.matmul(out=psf[:], lhsT=ones1[0:1, :], rhs=bo_sb[0:1, :],
                                 start=True, stop=False)
                nc.tensor.matmul(out=psf[:], lhsT=outT[:, 0, :], rhs=wo_sb[:, 0, :],
                                 start=False, stop=False)
                nc.tensor.matmul(out=psf[:], lhsT=outT[:, 1, :], rhs=wo_sb[:, 1, :],
                                 start=False, stop=True)
                nc.scalar.copy(out=outf_all[:, t, :], in_=psf[:])

            # ---------- emission = global schedule ----------
            load_xc(0)
            load_xc(1)
            emitted_g = 0
            while emitted_g <= dep_groups[0] and emitted_g < nwg:
                emit_wgroup(emitted_g)
                emitted_g += 1
            q_sb = emit_phase_b()
            for t in range(NT):
                while emitted_g <= dep_groups[t] and emitted_g < nwg:
                    emit_wgroup(emitted_g)
                    emitted_g += 1
                emit_gather(t)
                if t >= 1:
                    emit_qk(t - 1, q_sb)
                    emit_exp(t - 1)
                if t >= 2:
                    emit_post(t - 2)
                if t >= 3:
                    emit_pe(t - 3)
            while emitted_g < nwg:
                emit_wgroup(emitted_g)
                emitted_g += 1
            emit_qk(NT - 1, q_sb)
            emit_exp(NT - 1)
            emit_post(NT - 2)
            emit_pe(NT - 3)
            emit_post(NT - 1)
            emit_pe(NT - 2)
            emit_pe(NT - 1)
            nc.sync.dma_start(
                out=out_p[:].rearrange("(t p) c -> p t c", t=NT),
                in_=outf_all[:])

    nc.finalize()
    return nc


def _prep(x, adj, msk, Wq, Wkv, Wo, bo, null_k, null_v):
    """All host-side numpy prep.

    Returns (a_ts, dep_groups, kv_tiles, in_maps, order)."""
    deg = msk.sum(1).astype(np.int64)
    order = np.concatenate([
        np.full(NPAD - N, -1, dtype=np.int64),
        np.argsort(deg, kind="stable"),
    ])

    a_by_group = []
    for g in range(NT):
        grp = order[g * GROUP:(g + 1) * GROUP]
        real = grp[grp >= 0]
        mx = int(deg[real].max()) if real.size else 0
        a_by_group.append(max(mx, 1))
    # emission order: ascending degree, so early gathers are small (cheap
    # while phase A still owns the DMA engines) and need short prefixes.
    group_order = list(range(NT))
    a_ts = [a_by_group[g] for g in group_order]

    # compact each node's neighbor list: valid entries first
    sortcols = np.argsort(~msk, axis=1, kind="stable")
    comp = np.take_along_axis(adj, sortcols, axis=1)

    # permute v columns of Wkv (and null_v) from (h, dh) to (dh, h) order;
    # permute Wo rows to match.
    vperm = (np.arange(H)[None, :] * DH
             + np.arange(DH)[:, None]).reshape(-1)   # (d,h) -> h*DH+d
    Wkv2 = np.concatenate([Wkv[:, :HD], Wkv[:, HD:][:, vperm]], axis=1)
    Wo2 = Wo[vperm, :]
    nv2 = null_v.T.reshape(-1)                        # (d,h) flat
    scale = DH ** -0.5

    wq_h = np.ascontiguousarray(
        (Wq * scale).reshape(2, P, HD).transpose(1, 0, 2)).astype(BF)
    wkv_h = np.ascontiguousarray(
        Wkv2.reshape(2, P, KVW).transpose(1, 0, 2)).astype(BF)
    wo_h = np.ascontiguousarray(
        Wo2.reshape(2, P, D).transpose(1, 0, 2)).astype(BF)
    bo_h = bo.reshape(1, D).astype(BF)
    nullkv_h = np.tile(
        np.concatenate([null_k.reshape(-1), nv2]).reshape(1, KVW),
        (P, 1)).astype(BF)
    ident_h = np.eye(P, dtype=np.float32).astype(BF)

    # ---- per-core demand-ordered kv table ----
    # collect per-core, per-tile index blocks (original row ids), build the
    # first-use permutation, remap blocks, and compute per-tile max rows.
    core_blocks = []     # [core][tile] -> [128, a] remapped int array
    core_perm = []       # [core] -> original row ids in demand order
    used_counts = []
    for c in range(NCORES):
        blocks = []
        seen = np.zeros(N, bool)
        perm_parts = []
        for t, g in enumerate(group_order):
            a = a_ts[t]
            nodes = order[g * GROUP + c * P: g * GROUP + (c + 1) * P]
            nn = np.maximum(nodes, 0)
            valid = (np.arange(a)[None, :] < deg[nn][:, None]) & (nodes >= 0)[:, None]
            blk = np.where(valid, comp[nn, :a], 0)   # [128, a] original ids
            blocks.append((blk, valid))
            # first-use rows for this tile, in any stable order
            tile_rows = np.unique(blk)
            fresh = tile_rows[~seen[tile_rows]]
            seen[fresh] = True
            perm_parts.append(fresh)
        perm = np.concatenate(perm_parts)
        inv = np.full(N, -1, np.int64)
        inv[perm] = np.arange(len(perm))
        # keep blocks in demand-row space here; the device-table index
        # remap (partition-major) needs kv_tiles, computed below.
        remapped = [(np.where(valid, inv[blk], -1), valid)
                    for (blk, valid) in blocks]
        core_blocks.append(remapped)
        core_perm.append(perm)
        used_counts.append(len(perm))

    kv_tiles = (max(used_counts) + P - 1) // P
    nkv = kv_tiles * P
    wsizes = _wgroup_sizes(kv_tiles)
    nwg = len(wsizes)
    wends = np.cumsum(wsizes) * P                    # row end per group

    # per-tile dep group = write group covering the max demand row
    # (group g covers demand rows [wends[g-1], wends[g]))
    tile_max = np.zeros(NT, np.int64)
    for c in range(NCORES):
        for t in range(NT):
            blk, _ = core_blocks[c][t]
            tile_max[t] = max(tile_max[t], int(blk.max()) + 1)
    dep_groups = tuple(int(np.searchsorted(wends, m)) for m in tile_max)
    dep_groups = tuple(min(dgi, nwg - 1) for dgi in dep_groups)

    # partition-major table index: demand row r lives at (p, j) =
    # (r % 128, r // 128) -> gather idx p * kv_tiles + j.  Invalid/null
    # slots gather the last table row (the null kv row).
    def _to_idx(blk):
        return np.where(blk >= 0,
                        (blk % P) * kv_tiles + blk // P,
                        P * kv_tiles)

    in_maps = []
    for c in range(NCORES):
        # demand-ordered x, padded to nkv rows
        xo = np.zeros((nkv, D), np.float32)
        xo[:used_counts[c]] = x[core_perm[c]]
        xT_h = np.ascontiguousarray(
            xo.T.reshape(2, P, kv_tiles, P).transpose(1, 2, 0, 3)).astype(BF)

        xp = np.zeros((NT * P, D), np.float32)
        flats = []
        mblocks = []
        for t, g in enumerate(group_order):
            a = a_ts[t]
            nodes = order[g * GROUP + c * P: g * GROUP + (c + 1) * P]
            xp[t * P:(t + 1) * P][nodes >= 0] = x[nodes[nodes >= 0]]
            blk, valid = core_blocks[c][t]
            # slot 0 = null row (last table row) for every node
            blk16 = np.concatenate(
                [np.full((P, 1), P * kv_tiles, np.int16),
                 _to_idx(blk).astype(np.int16)], axis=1)
            flats.append(blk16.T.reshape(-1))        # i = col*128+p
            m = np.zeros((P, 1 + a), np.float32)
            m[:, 0] = 1.0
            m[:, 1:] = valid
            mblocks.append(m)
        flat = np.concatenate(flats)
        idx_h = np.ascontiguousarray(
            np.tile(flat.reshape(-1, 16).T, (8, 1))).astype(np.int16)
        mask_h = np.ascontiguousarray(np.concatenate(mblocks, axis=1))
        xpT_h = np.ascontiguousarray(
            xp.T.reshape(2, P, NT * P).transpose(1, 0, 2)).astype(BF)
        in_maps.append({
            "xT": xT_h, "xpT": xpT_h, "wq": wq_h, "wkv": wkv_h, "wo": wo_h,
            "bo": bo_h, "nullkv": nullkv_h, "ident": ident_h,
            "idxs": idx_h, "masks": mask_h,
        })
    return a_ts, dep_groups, kv_tiles, in_maps, order


def kernel(x, adj_kv_indices, mask, Wq, Wkv, Wo, bo, null_k, null_v):
    global LAST_EXEC_NS
    x = np.asarray(x, dtype=np.float32)[0]
    adj = np.asarray(adj_kv_indices)[0].astype(np.int64)
    msk = np.asarray(mask)[0].astype(bool)
    Wq = np.asarray(Wq, np.float32)
    Wkv = np.asarray(Wkv, np.float32)
    Wo = np.asarray(Wo, np.float32)
    bo = np.asarray(bo, np.float32)
    null_k = np.asarray(null_k, np.float32)
    null_v = np.asarray(null_v, np.float32)

    a_ts, dep_groups, kv_tiles, in_maps, order = _prep(
        x, adj, msk, Wq, Wkv, Wo, bo, null_k, null_v)
    nc = _build(tuple(a_ts), dep_groups, kv_tiles)
    res = run_bass_kernel_spmd(
        nc, in_maps, core_ids=list(range(NCORES)),
        trace=bool(os.environ.get("KERNEL_TRACE")))
    LAST_EXEC_NS = res.exec_time_ns

    group_order = list(range(NT))
    out_full = np.zeros((N, D), np.float32)
    for c in range(NCORES):
        o = np.asarray(res.results[c]["out"])
        for t, g in enumerate(group_order):
            nodes = order[g * GROUP + c * P: g * GROUP + (c + 1) * P]
            sel = nodes >= 0
            out_full[nodes[sel]] = o[t * P:(t + 1) * P][sel]
    return out_full.reshape(1, N, D)
